# revision 1
# baseline (speedup 1.0000x reference)
"""Trainium2 Bass kernel for nn_Attention (B=2, N=2048, C=768, H=12, D=64).

Sharding: 8 cores = 2 batches x 4 head-groups (3 heads each).
Per core: full attention for its (batch, 3 heads) + row-sharded proj
partial output [2048, 768]; host sums the 4 partials per batch (+b_proj).

Layout strategy (per core):
  xT [768, 2048] via PE transposes (x arrives [2048, 768]).
  qkvT = W_slice.T @ xT, with M-tiles packed for row-tiled score pairs:
      T0=[qT_h0;qT_h1] T1=[kT_h0;kT_h1] T2=[qT_h2;qT_h2]
      T3=[kT_h2;kT_h2] T4=[vT_h0;vT_h1] T5=[vT_h2]
  Scores computed TRANSPOSED (sT[k, q] = kT.T @ qT) so softmaxed probs are
  directly the PV rhs (no P transposes). exp on ACT with scale=0.125 folded
  in, no max subtraction (scores ~ N(0,1)).  PV matmuls are M=65 with an
  appended ones-column: row 64 of each accumulator is the softmax
  denominator, for free.  Normalization: aligned reciprocal (DVE, row 64)
  -> ones-matmul partition-broadcast from contraction-row 64 -> DVE mul.
  All engine ops keep matching partition bases (partition-shifted DVE ops
  and quadrant-3 (partitions >=96) col-tiled matmul dsts crash/fail TRN2).
  Matmul dtype: float32r everywhere (full-rate fp32 mode, ~1.6e-4 rel err).
"""

import numpy as np

import concourse.bass as bass
import concourse.mybir as mybir
from concourse import bacc, tile
from concourse.bass_utils import run_bass_kernel_spmd
from concourse.masks import make_identity

F32 = mybir.dt.float32
F32R = mybir.dt.float32r
BF16 = mybir.dt.bfloat16
AF = mybir.ActivationFunctionType

B, N, C = 2, 2048, 768
H, D = 12, 64
SCALE = D ** -0.5  # 0.125
NCORES = 8
HPC = 3            # heads per core
NK = N // 128      # 16 k-tiles
NQ4 = N // 512     # 4 q-chunks of 512
WM = 704           # packed qkv weight columns: 5*128 + 64


def build_program():
    nc = bacc.Bacc("TRN2", target_bir_lowering=False, debug=False,
                   num_devices=NCORES)
    x_d = nc.dram_tensor("x", [N, C], F32, kind="ExternalInput")
    w_d = nc.dram_tensor("w", [C, WM], F32, kind="ExternalInput")
    bq_d = nc.dram_tensor("bq", [128, 6], F32, kind="ExternalInput")
    wp_d = nc.dram_tensor("wp", [HPC, 64, C], F32, kind="ExternalInput")
    y_d = nc.dram_tensor("y", [N, C], F32, kind="ExternalOutput")

    CT = C // 128  # 6 c-tiles

    with tile.TileContext(nc) as tc:
        with (
            tc.tile_pool(name="const", bufs=1) as cpool,
            tc.tile_pool(name="wr", bufs=1) as wrpool,
            tc.tile_pool(name="qkT", bufs=1) as qkpool,
            tc.tile_pool(name="vn", bufs=1) as vnpool,
            tc.tile_pool(name="outT", bufs=1) as opool,
        ):
            ident = cpool.tile([128, 128], F32)
            make_identity(nc, ident[:])
            ones_f = cpool.tile([65, 64], F32)
            nc.gpsimd.memset(ones_f[:], 1.0)
            ones_hi = cpool.tile([65, 64], F32R)  # row 64: bcast lhsT
            nc.vector.tensor_copy(ones_hi[:], ones_f[:])
            vcol_f = cpool.tile([128, NK, 1], F32)
            nc.gpsimd.memset(vcol_f[:], 1.0)
            bq_sb = cpool.tile([128, 6], F32)
            nc.sync.dma_start(out=bq_sb[:], in_=bq_d[:])

            w_r = wrpool.tile([128, CT, WM], F32R)
            wp_r = wrpool.tile([64, HPC, C], F32R)

            qkT = [qkpool.tile([128, N], BF16, tag=f"qkT{t}", name=f"qkT{t}")
                   for t in range(4)]
            v_n = [vnpool.tile([128, NK, 128], BF16, tag=f"vn{h}", name=f"vn{h}")
                   for h in range(HPC)]
            outT = [opool.tile([64, N], F32R, tag=f"outT{h}", name=f"outT{h}")
                    for h in range(HPC)]

            # ---------------- Phase 1: loads, xT, qkvT, v_n ----------------
            with (
                tc.tile_pool(name="stage", bufs=1) as spool,
                tc.tile_pool(name="xraw", bufs=2) as xpool,
                tc.tile_pool(name="p1ps", bufs=4, space="PSUM") as tppool,
                tc.tile_pool(name="qkvps", bufs=4, space="PSUM") as qpspool,
            ):
                xT = [spool.tile([128, N], F32R, tag=f"xT{t}", name=f"xT{t}")
                      for t in range(CT)]
                vT4 = spool.tile([128, N], F32)
                vT5 = spool.tile([64, N], F32)
                xr0 = xpool.tile([128, 4, C], F32, tag="xraw", name="xr0")
                nc.sync.dma_start(
                    out=xr0[:], in_=x_d[0:512, :].rearrange("(j p) c -> p j c", p=128))
                w_sb = spool.tile([128, CT, WM], F32)
                nc.gpsimd.dma_start(
                    out=w_sb[:], in_=w_d.ap().rearrange("(t p) m -> p t m", p=128))
                for t in range(CT):
                    nc.vector.tensor_copy(w_r[:, t, :], w_sb[:, t, :])
                wp_sb = spool.tile([64, HPC, C], F32)
                nc.gpsimd.dma_start(
                    out=wp_sb[:], in_=wp_d.ap().rearrange("h p c -> p h c"))
                nc.vector.tensor_copy(wp_r[:], wp_sb[:])

                def qkv_tile(t, nch):
                    ns = slice(nch * 512, (nch + 1) * 512)
                    m0, m1 = t * 128, min((t + 1) * 128, WM)
                    mm = m1 - m0
                    qps = qpspool.tile([128, 512], F32, tag="qkv",
                                       name=f"qps{t}_{nch}")
                    for ct in range(CT):
                        nc.tensor.matmul(qps[0:mm, :], w_r[:, ct, m0:m1],
                                         xT[ct][:, ns], start=(ct == 0),
                                         stop=(ct == CT - 1))
                    bias = bq_sb[:, t:t + 1] if mm == 128 else bq_sb[0:mm, t:t + 1]
                    if t < 4:
                        dst = qkT[t][:, ns]
                    elif t == 4:
                        dst = vT4[:, ns]
                    else:
                        dst = vT5[0:64, ns]
                    nc.vector.tensor_scalar(dst, qps[0:mm, :], bias, None,
                                            mybir.AluOpType.add)

                for h in range(HPC):
                    nc.gpsimd.memset(v_n[h][:], 0.0)
                vsrc = [(vT4[0:64, :], ident[0:64, 0:64]),
                        (vT4[64:128, :], ident[64:128, 64:128]),
                        (vT5[0:64, :], ident[0:64, 0:64])]
                for nch in range(NQ4):          # 512-row chunks
                    ns = slice(nch * 512, (nch + 1) * 512)
                    if nch == 0:
                        xr = xr0
                    else:
                        xr = xpool.tile([128, 4, C], F32, tag="xraw",
                                        name=f"xr{nch}")
                        nc.sync.dma_start(
                            out=xr[:],
                            in_=x_d[ns, :].rearrange("(j p) c -> p j c", p=128))
                    for ct in range(CT):
                        tp = tppool.tile([128, 512], F32, tag="tp")
                        for j in range(4):
                            nc.tensor.transpose(tp[:, j * 128:(j + 1) * 128],
                                                xr[:, j, ct * 128:(ct + 1) * 128],
                                                ident[:])
                        nc.vector.tensor_copy(xT[ct][:, ns], tp[:])
                    for t in (1, 0, 3, 2, 4, 5):
                        qkv_tile(t, nch)
                    for h in range(HPC):
                        srcv, idn = vsrc[h]
                        tp = tppool.tile([128, 256], F32, tag="tp")
                        for j in range(4):
                            k = nch * 4 + j
                            nc.tensor.transpose(tp[:, j * 64:(j + 1) * 64],
                                                srcv[:, k * 128:(k + 1) * 128], idn)
                        nc.vector.tensor_copy(
                            v_n[h][:, nch * 4:(nch + 1) * 4, 0:64],
                            tp[:].rearrange("p (j d) -> p j d", j=4))
                    if nch == 0:
                        for h in range(HPC):
                            nc.vector.tensor_copy(v_n[h][:, :, 64:65], vcol_f[:])

            # ------------- Phase 2+3: attention + proj, interleaved -------------
            with (
                tc.tile_pool(name="accps", bufs=1, space="PSUM") as acpool,
                tc.tile_pool(name="pjps", bufs=1, space="PSUM") as pjpool,
                tc.tile_pool(name="scps", bufs=2, space="PSUM") as scpool,
                tc.tile_pool(name="pt", bufs=24) as ptpool,
                tc.tile_pool(name="pt2", bufs=12) as ptpool2,
                tc.tile_pool(name="rc", bufs=4) as rcpool,
                tc.tile_pool(name="y", bufs=2) as ypool,
                tc.tile_pool(name="dr", bufs=4, space="DRAM") as drpool,
            ):
                def norm_apply(acc, dsts, rtag):
                    """recip of acc row 64 -> DMA partition-bcast -> mul."""
                    with nc.allow_low_precision(reason="f32r recip"):
                        r = rcpool.tile([65, 512], F32R, tag=rtag, name=rtag)
                        nc.vector.reciprocal(r[64:65, :], acc[64:65, :])
                    rd = drpool.tile([1, 512], F32R, tag="rd", name="rd")
                    nc.gpsimd.dma_start(out=rd[:], in_=r[64:65, :])
                    bcs = rcpool.tile([64, 512], F32R, tag="bcs", name="bcs")
                    bcast_ap = bass.AP(tensor=rd.tensor, offset=rd.offset,
                                       ap=[[0, 64]] + list(rd.ap()[1:] if callable(getattr(rd, "ap", None)) else rd[:].ap[1:]))
                    nc.gpsimd.dma_start(out=bcs[:], in_=bcast_ap)
                    nc.vector.tensor_mul(dsts, acc[0:64, :], bcs[:])

                def pair_scores(qc):
                    qs = slice(qc * 512, (qc + 1) * 512)
                    pts = []
                    for k in range(NK):
                        ks = slice(k * 128, (k + 1) * 128)
                        sc = scpool.tile([128, 1024], F32, tag="scores", name="sc")
                        nc.tensor.matmul(sc[:, 0:512], qkT[1][0:64, ks],
                                         qkT[0][0:64, qs], start=True, stop=True)
                        nc.tensor.matmul(sc[:, 512:1024], qkT[1][64:128, ks],
                                         qkT[0][64:128, qs], start=True, stop=True,
                                         tile_position=(64, 0))
                        pt = ptpool.tile([128, 1024], BF16, tag="pt", name="pt")
                        nc.scalar.activation(pt[:], sc[:], AF.Exp, scale=SCALE)
                        pts.append(pt)
                    return pts

                def h2_scores(qc):
                    qs = slice(qc * 512, (qc + 1) * 512)
                    pts = []
                    for kp in range(NK // 2):
                        ke = slice((2 * kp) * 128, (2 * kp + 1) * 128)
                        ko = slice((2 * kp + 1) * 128, (2 * kp + 2) * 128)
                        sc = scpool.tile([128, 1024], F32, tag="scores", name="sc")
                        nc.tensor.matmul(sc[:, 0:512], qkT[3][0:64, ke],
                                         qkT[2][0:64, qs], start=True, stop=True)
                        nc.tensor.matmul(sc[:, 512:1024], qkT[3][64:128, ko],
                                         qkT[2][64:128, qs], start=True, stop=True,
                                         tile_position=(64, 0))
                        pt = ptpool2.tile([128, 1024], BF16, tag="pt2", name="pt2")
                        nc.scalar.activation(pt[:], sc[:], AF.Exp, scale=SCALE)
                        pts.append(pt)
                    return pts

                def pair_pv(qc, pts):
                    qs = slice(qc * 512, (qc + 1) * 512)
                    s1 = acpool.tile([128, 512], F32, tag="s1", bufs=3, name="s1")
                    s2 = acpool.tile([128, 512], F32, tag="s1", bufs=3, name="s2")
                    for k in range(NK):
                        nc.tensor.matmul(s2[:, :], v_n[1][:, k, :],
                                         pts[k][:, 512:1024],
                                         start=(k == 0), stop=(k == NK - 1))
                    norm_apply(s2, outT[1][0:64, qs], "r1")
                    for k in range(NK):
                        nc.tensor.matmul(s1[:, :], v_n[0][:, k, :],
                                         pts[k][:, 0:512],
                                         start=(k == 0), stop=(k == NK - 1))
                    norm_apply(s1, outT[0][0:64, qs], "r0")

                def h2_pv(qc, pts):
                    qs = slice(qc * 512, (qc + 1) * 512)
                    s3 = acpool.tile([128, 512], F32, tag="s1", bufs=3, name="s3")
                    for kp in range(NK // 2):
                        nc.tensor.matmul(s3[:, :], v_n[2][:, 2 * kp, :],
                                         pts[kp][:, 0:512],
                                         start=(kp == 0), stop=False)
                    for kp in range(NK // 2):
                        nc.tensor.matmul(s3[:, :], v_n[2][:, 2 * kp + 1, :],
                                         pts[kp][:, 512:1024], start=False,
                                         stop=(kp == NK // 2 - 1))
                    norm_apply(s3, outT[2][0:64, qs], "r0")

                def proj(qc):
                    for j in range(4):
                        qj = slice(qc * 512 + j * 128, qc * 512 + (j + 1) * 128)
                        y_sb = ypool.tile([128, C], F32, tag="y", name="ysb")
                        pj = pjpool.tile([128, 512], F32, tag="proj", bufs=1, name="pj")
                        for h in range(HPC):
                            nc.tensor.matmul(pj[:, 0:512], outT[h][0:64, qj],
                                             wp_r[0:64, h, 0:512],
                                             start=(h == 0), stop=(h == HPC - 1))
                        nc.vector.tensor_copy(y_sb[:, 0:512], pj[:, 0:512])
                        pj2 = pjpool.tile([128, 512], F32, tag="proj", bufs=1, name="pj2")
                        for h in range(HPC):
                            nc.tensor.matmul(pj2[:, 0:256], outT[h][0:64, qj],
                                             wp_r[0:64, h, 512:768],
                                             start=(h == 0), stop=(h == HPC - 1))
                        nc.vector.tensor_copy(y_sb[:, 512:768], pj2[:, 0:256])
                        nc.sync.dma_start(out=y_d[qj, :], in_=y_sb[:])

                for qc in range(NQ4):
                    p_pts = pair_scores(qc)
                    h_pts = h2_scores(qc)
                    if qc > 0:
                        proj(qc - 1)
                    pair_pv(qc, p_pts)
                    h2_pv(qc, h_pts)
                proj(NQ4 - 1)

    nc.compile()
    return nc


def make_in_maps(x, w_qkv, b_qkv, w_proj):
    """Per-core input dicts. Core c: batch c//4, heads 3*(c%4)+[0..2]."""
    x = np.asarray(x, np.float32)
    w_qkv = np.asarray(w_qkv, np.float32)
    b_qkv = np.asarray(b_qkv, np.float32)
    w_proj = np.asarray(w_proj, np.float32)
    q = lambda h: w_qkv[:, h * 64:(h + 1) * 64]
    k = lambda h: w_qkv[:, C + h * 64: C + (h + 1) * 64]
    v = lambda h: w_qkv[:, 2 * C + h * 64: 2 * C + (h + 1) * 64]
    qb = lambda h: b_qkv[h * 64:(h + 1) * 64]
    kb = lambda h: b_qkv[C + h * 64: C + (h + 1) * 64]
    vb = lambda h: b_qkv[2 * C + h * 64: 2 * C + (h + 1) * 64]
    in_maps = []
    for c in range(NCORES):
        b = c // 4
        h0 = 3 * (c % 4)
        h1, h2 = h0 + 1, h0 + 2
        w_pack = np.concatenate(
            [q(h0), q(h1), k(h0), k(h1), q(h2), q(h2), k(h2), k(h2),
             v(h0), v(h1), v(h2)], axis=1).astype(np.float32)
        bias = np.concatenate(
            [qb(h0), qb(h1), kb(h0), kb(h1), qb(h2), qb(h2), kb(h2), kb(h2),
             vb(h0), vb(h1), vb(h2), np.zeros(64, np.float32)])
        bq_pack = bias.reshape(6, 128).T.copy()  # [128, 6]
        wp_pack = np.stack([w_proj[h * 64:(h + 1) * 64, :] for h in (h0, h1, h2)])
        in_maps.append({
            "x": np.ascontiguousarray(x[b]),
            "w": np.ascontiguousarray(w_pack),
            "bq": np.ascontiguousarray(bq_pack),
            "wp": np.ascontiguousarray(wp_pack),
        })
    return in_maps


_NC_CACHE = []


def _get_program():
    if not _NC_CACHE:
        _NC_CACHE.append(build_program())
    return _NC_CACHE[0]


def run(inputs, trace=False, **kw):
    nc = _get_program()
    in_maps = make_in_maps(inputs["x"], inputs["w_qkv"], inputs["b_qkv"],
                           inputs["w_proj"])
    res = run_bass_kernel_spmd(nc, in_maps, list(range(NCORES)), trace=trace, **kw)
    b_proj = np.asarray(inputs["b_proj"], np.float32)
    out = np.zeros((B, N, C), np.float32)
    for c in range(NCORES):
        out[c // 4] += res.results[c]["y"]
    out += b_proj[None, None, :]
    return out.astype(np.float32), res


def kernel(**inputs):
    out, _ = run(inputs)
    return out



# revision 12
# speedup vs baseline: 1.0273x; 1.0273x over previous
"""Trainium2 Bass kernel for nn_Attention (B=2, N=2048, C=768, H=12, D=64).

Sharding: 8 cores = 2 batches x 4 head-groups (3 heads each).
Per core: full attention for its (batch, 3 heads) + row-sharded proj
partial output [2048, 768]; host sums the 4 partials per batch (+b_proj).

v2 design (ACT-saturation + p-stationary PV):
  The exp on the ACT engine (12.6M score elems/core at 1 elem/lane/cy
  @1.2GHz) is a hard ~100us floor; everything else is scheduled to hide
  under it.
  - x/w in bf16; x chunks of 256 tokens: PE-transpose -> xT (bf16),
    K matmuls per chunk so the first exp launches ~6us in; Q per
    q-chunk and V per k-pair are interleaved into the attention loop.
  - Scores transposed (sT[k,q] = kT.T @ qT), row-tiled pairs at
    tile_position (0,0)/(64,0) writing DIFFERENT PSUM banks (scA/scB)
    so the two 64-contraction matmuls can overlap. exp in [128,1024]
    ACTIVATEs with scale=0.125 folded in, no max subtraction.
  - PV p-stationary: lhsT = pt[:, q-tile] (128 cols bf16 -> FWL),
    rhs = [v | ones] (N=65): out acc[q,65] accumulates over k; col 64
    is the softmax denominator -> reciprocal_approx_fast on [128,1] +
    per-partition tensor_scalar mult (replaces 40us of 1-lane DVE
    reciprocals + DMA broadcasts in v1).
  - Proj: normalized o written as [q, h0d|h1d] + [q, h2d]; one PE
    transpose stacks h0/h1 on 128 partitions -> proj = K=128 + K=64
    accumulated matmuls per q-tile.
  PSUM: scA/scB 2 banks each, accA/accB 1 each, aux rotation 2 = 8.
"""

import numpy as np
import ml_dtypes

import concourse.bass as bass
import concourse.mybir as mybir
from concourse import bacc, tile
from concourse.bass_utils import run_bass_kernel_spmd
from concourse.masks import make_identity

F32 = mybir.dt.float32
F32R = mybir.dt.float32r
BF16 = mybir.dt.bfloat16
AF = mybir.ActivationFunctionType
ADD = mybir.AluOpType.add
MULT = mybir.AluOpType.mult

B, N, C = 2, 2048, 768
H, D = 12, 64
SCALE = D ** -0.5  # 0.125
NCORES = 8
HPC = 3            # heads per core
NK = N // 128      # 16 k-tiles
NQC = N // 512     # 4 q-chunks
NCH = N // 256     # 8 x chunks
CT = C // 128      # 6 c-tiles


def build_program():
    nc = bacc.Bacc("TRN2", target_bir_lowering=False, debug=False,
                   num_devices=NCORES)
    x_d = nc.dram_tensor("x", [N, C], F32, kind="ExternalInput")
    w_d = nc.dram_tensor("w", [C, 512], BF16, kind="ExternalInput")
    wv_d = nc.dram_tensor("wv", [C, 192], BF16, kind="ExternalInput")
    bq_d = nc.dram_tensor("bq", [128, 4], F32, kind="ExternalInput")
    vb_d = nc.dram_tensor("vb", [128, 192], F32, kind="ExternalInput")
    wp01_d = nc.dram_tensor("wp01", [128, C], F32, kind="ExternalInput")
    wp2_d = nc.dram_tensor("wp2", [64, C], F32, kind="ExternalInput")
    y_d = nc.dram_tensor("y", [N, C], F32, kind="ExternalOutput")

    with tile.TileContext(nc) as tc:
        with (
            tc.tile_pool(name="const", bufs=1) as cpool,
            tc.tile_pool(name="w", bufs=1) as wpool,
            tc.tile_pool(name="xr", bufs=2) as xpool,
            tc.tile_pool(name="xT", bufs=1) as spool,
            tc.tile_pool(name="qk", bufs=1) as qkpool,
            tc.tile_pool(name="v", bufs=1) as vpool,
            tc.tile_pool(name="pt", bufs=8) as ptpool,
            tc.tile_pool(name="o", bufs=1) as opool,
            tc.tile_pool(name="y", bufs=2) as ypool,
            tc.tile_pool(name="r", bufs=4) as rpool,
            tc.tile_pool(name="sc", bufs=1, space="PSUM") as scpool,
            tc.tile_pool(name="acc", bufs=1, space="PSUM") as acpool,
            tc.tile_pool(name="aux", bufs=2, space="PSUM") as auxpool,
        ):
            identF = cpool.tile([128, 128], F32)
            make_identity(nc, identF[:])
            bq_sb = cpool.tile([128, 4], F32)
            nc.sync.dma_start(out=bq_sb[:], in_=bq_d[:])
            vb_sb = cpool.tile([128, 192], F32)
            nc.gpsimd.dma_start(out=vb_sb[:], in_=vb_d[:])

            w_r = wpool.tile([128, CT, 512], BF16)
            nc.gpsimd.dma_start(
                out=w_r[:], in_=w_d.ap().rearrange("(t p) m -> p t m", p=128))
            wv_r = wpool.tile([128, CT, 192], BF16)
            nc.gpsimd.dma_start(
                out=wv_r[:], in_=wv_d.ap().rearrange("(t p) m -> p t m", p=128))
            wp01_f = wpool.tile([128, C], F32)
            nc.gpsimd.dma_start(out=wp01_f[:], in_=wp01_d[:])
            wp01_r = wpool.tile([128, C], F32R)
            nc.vector.tensor_copy(wp01_r[:], wp01_f[:])
            wp2_f = wpool.tile([64, C], F32)
            nc.gpsimd.dma_start(out=wp2_f[:], in_=wp2_d[:])
            wp2_r = wpool.tile([64, C], F32R)
            nc.vector.tensor_copy(wp2_r[:], wp2_f[:])

            xT = [spool.tile([128, N], BF16, tag=f"xT{t}", name=f"xT{t}")
                  for t in range(CT)]
            qT01 = qkpool.tile([128, N], BF16, tag="qT01", name="qT01")
            kT01 = qkpool.tile([128, N], BF16, tag="kT01", name="kT01")
            qT22 = qkpool.tile([128, N], BF16, tag="qT22", name="qT22")
            kT22 = qkpool.tile([128, N], BF16, tag="kT22", name="kT22")
            # v layout: [keys, k-tile, h0 d(64)+1 | h1 d+1 | h2 d+1] (3*65)
            vall = vpool.tile([128, NK, 195], BF16, tag="vall", name="vall")
            for h in range(HPC):
                nc.gpsimd.memset(vall[:, :, h * 65 + 64:h * 65 + 65], 1.0)

            accA = acpool.tile([128, 512], F32, tag="accA", name="accA")
            accB = acpool.tile([128, 512], F32, tag="accB", name="accB")

            def chain_slice(h, qj):
                """PSUM (tile, col) of PV accumulator for chain (head, qj)."""
                if h == 0:
                    return accA, qj * 65
                if h == 1:
                    return (accA, 260 + qj * 65) if qj < 2 else (accB, (qj - 2) * 65)
                return accB, 130 + qj * 65

            def chunk(ch):
                """Load 256 tokens, transpose into xT, K matmuls -> kT01/kT22."""
                xr = xpool.tile([128, 2, C], F32, tag="xr", name=f"xr{ch}")
                nc.sync.dma_start(
                    out=xr[:],
                    in_=x_d[ch * 256:(ch + 1) * 256, :].rearrange(
                        "(j p) c -> p j c", p=128))
                ns = slice(ch * 256, (ch + 1) * 256)
                for pr in range(3):
                    tp = auxpool.tile([128, 512], F32, tag="aux",
                                      name=f"tp{ch}_{pr}")
                    for i in range(2):
                        ct = 2 * pr + i
                        for j in range(2):
                            nc.tensor.transpose(
                                tp[:, (i * 2 + j) * 128:(i * 2 + j + 1) * 128],
                                xr[:, j, ct * 128:(ct + 1) * 128], identF[:])
                    nc.vector.tensor_copy(xT[2 * pr][:, ns], tp[:, 0:256])
                    nc.vector.tensor_copy(xT[2 * pr + 1][:, ns], tp[:, 256:512])
                kp = auxpool.tile([128, 512], F32, tag="aux", name=f"kp{ch}")
                for i, t in enumerate((1, 3)):
                    for ct in range(CT):
                        nc.tensor.matmul(kp[:, i * 256:(i + 1) * 256],
                                         w_r[:, ct, t * 128:(t + 1) * 128],
                                         xT[ct][:, ns], start=(ct == 0),
                                         stop=(ct == CT - 1))
                nc.vector.tensor_scalar(kT01[:, ns], kp[:, 0:256],
                                        bq_sb[:, 1:2], None, ADD)
                nc.vector.tensor_scalar(kT22[:, ns], kp[:, 256:512],
                                        bq_sb[:, 3:4], None, ADD)

            def q_block(qc):
                ns = slice(qc * 512, (qc + 1) * 512)
                for t, dst, bcol in ((0, qT01, 0), (2, qT22, 2)):
                    qp = auxpool.tile([128, 512], F32, tag="aux",
                                      name=f"qp{qc}_{t}")
                    for ct in range(CT):
                        nc.tensor.matmul(qp[:], w_r[:, ct, t * 128:(t + 1) * 128],
                                         xT[ct][:, ns], start=(ct == 0),
                                         stop=(ct == CT - 1))
                    nc.vector.tensor_scalar(dst[:, ns], qp[:],
                                            bq_sb[:, bcol:bcol + 1], None, ADD)

            def v_block(k0):
                """v for k-tiles k0, k0+1 -> vall (with bias add)."""
                vp = auxpool.tile([128, 512], F32, tag="aux", name=f"vp{k0}")
                for i in range(2):
                    ts = slice((k0 + i) * 128, (k0 + i + 1) * 128)
                    for ct in range(CT):
                        nc.tensor.matmul(vp[:, i * 256:i * 256 + 192],
                                         xT[ct][:, ts], wv_r[:, ct, :],
                                         start=(ct == 0), stop=(ct == CT - 1))
                for i in range(2):
                    src = vp[:, i * 256:i * 256 + 192].rearrange(
                        "p (h x) -> p h x", h=3)
                    dst = vall[:, k0 + i, :].rearrange(
                        "p (h x) -> p h x", h=3)[:, :, 0:64]
                    nc.vector.tensor_add(dst, src,
                                         vb_sb[:].rearrange("p (h x) -> p h x", h=3))

            def scores_pair(qc, j):
                """Heads 0/1, k-tiles 2j, 2j+1 -> exp'd pt tiles."""
                qs = slice(qc * 512, (qc + 1) * 512)
                scA = scpool.tile([128, 1024], F32, tag="scA", name="scA")
                scB = scpool.tile([128, 1024], F32, tag="scB", name="scB")
                for kk in range(2):
                    ks = slice((2 * j + kk) * 128, (2 * j + kk + 1) * 128)
                    nc.tensor.matmul(scA[:, kk * 512:(kk + 1) * 512],
                                     kT01[0:64, ks], qT01[0:64, qs],
                                     start=True, stop=True)
                    nc.tensor.matmul(scB[:, kk * 512:(kk + 1) * 512],
                                     kT01[64:128, ks], qT01[64:128, qs],
                                     start=True, stop=True, tile_position=(64, 0))
                ptA = ptpool.tile([128, 1024], BF16, tag="ptA", name="ptA")
                ptB = ptpool.tile([128, 1024], BF16, tag="ptB", name="ptB")
                nc.scalar.activation(ptA[:], scA[:], AF.Exp, scale=SCALE)
                nc.scalar.activation(ptB[:], scB[:], AF.Exp, scale=SCALE)
                return ptA, ptB

            def scores_h2(qc, i):
                """Head 2: scA gets k=4i,4i+2 (rt0); scB k=4i+1,4i+3 (rt1)."""
                qs = slice(qc * 512, (qc + 1) * 512)
                scA = scpool.tile([128, 1024], F32, tag="scA", name="scA")
                scB = scpool.tile([128, 1024], F32, tag="scB", name="scB")
                for kk in range(2):
                    kA = 4 * i + 2 * kk
                    kB = kA + 1
                    nc.tensor.matmul(scA[:, kk * 512:(kk + 1) * 512],
                                     kT22[0:64, kA * 128:(kA + 1) * 128],
                                     qT22[0:64, qs], start=True, stop=True)
                    nc.tensor.matmul(scB[:, kk * 512:(kk + 1) * 512],
                                     kT22[64:128, kB * 128:(kB + 1) * 128],
                                     qT22[64:128, qs], start=True, stop=True,
                                     tile_position=(64, 0))
                ptA = ptpool.tile([128, 1024], BF16, tag="ptA", name="ptA2")
                ptB = ptpool.tile([128, 1024], BF16, tag="ptB", name="ptB2")
                nc.scalar.activation(ptA[:], scA[:], AF.Exp, scale=SCALE)
                nc.scalar.activation(ptB[:], scB[:], AF.Exp, scale=SCALE)
                return ptA, ptB

            def pv_pair_burst(ptsA, ptsB):
                """One chain (= one open PSUM accum group) at a time."""
                for h, pts in ((0, ptsA), (1, ptsB)):
                    for qj in range(4):
                        acc, c0 = chain_slice(h, qj)
                        for k in range(NK):
                            pt = pts[k // 2]
                            q0 = (k % 2) * 512 + qj * 128
                            nc.tensor.matmul(acc[:, c0:c0 + 65],
                                             pt[:, q0:q0 + 128],
                                             vall[:, k, h * 65:(h + 1) * 65],
                                             start=(k == 0), stop=(k == NK - 1))

            def pv_h2_burst(hA, hB):
                for qj in range(4):
                    acc, c0 = chain_slice(2, qj)
                    seq = []
                    for i in range(4):
                        seq += [(hA[i], 0, 4 * i), (hA[i], 1, 4 * i + 2),
                                (hB[i], 0, 4 * i + 1), (hB[i], 1, 4 * i + 3)]
                    for n, (pt, kk, k) in enumerate(seq):
                        q0 = kk * 512 + qj * 128
                        nc.tensor.matmul(acc[:, c0:c0 + 65], pt[:, q0:q0 + 128],
                                         vall[:, k, 130:195],
                                         start=(n == 0), stop=(n == NK - 1))

            o01s = [None] * 4
            o2s = [None] * 4

            def norm_qc(qc):
                """DVE: acc -> normalized o (reads acc BEFORE next qc's PV)."""
                for qj in range(4):
                    o01 = opool.tile([128, 128], F32, tag=f"o01_{qj}",
                                     name=f"o01_{qc}_{qj}")
                    o2 = opool.tile([128, 64], F32, tag=f"o2_{qj}",
                                    name=f"o2_{qc}_{qj}")
                    for h in range(HPC):
                        acc, c0 = chain_slice(h, qj)
                        r = rpool.tile([128, 1], F32, tag="r", name="r")
                        with nc.allow_low_precision(reason="softmax denom recip"):
                            nc.vector.reciprocal_approx_fast(
                                r[:], acc[:, c0 + 64:c0 + 65])
                        dst = o01[:, h * 64:(h + 1) * 64] if h < 2 else o2[:]
                        nc.vector.tensor_scalar(dst, acc[:, c0:c0 + 64],
                                                r[:], None, MULT)
                    o01s[qj] = o01
                    o2s[qj] = o2

            def proj_qc(qc):
                """PE: transpose-stack o, proj matmuls, y out."""
                for qj in range(4):
                    qrows = slice(qc * 512 + qj * 128, qc * 512 + (qj + 1) * 128)
                    tT = auxpool.tile([128, 512], F32, tag="aux",
                                      name=f"oT{qc}_{qj}")
                    nc.tensor.transpose(tT[:, 0:128], o01s[qj][:], identF[:])
                    nc.tensor.transpose(tT[0:64, 128:256], o2s[qj][:], identF[:])
                    oT01 = opool.tile([128, 128], F32R, tag="oT01", bufs=2,
                                      name="oT01")
                    oT2 = opool.tile([64, 128], F32R, tag="oT2", bufs=2,
                                     name="oT2")
                    nc.vector.tensor_copy(oT01[:], tT[:, 0:128])
                    nc.vector.tensor_copy(oT2[:], tT[0:64, 128:256])
                    ya = auxpool.tile([128, 512], F32, tag="aux",
                                      name=f"ya{qc}_{qj}")
                    nc.tensor.matmul(ya[:], oT01[:], wp01_r[:, 0:512],
                                     start=True, stop=False)
                    nc.tensor.matmul(ya[:], oT2[:], wp2_r[:, 0:512],
                                     start=False, stop=True)
                    yb = auxpool.tile([128, 512], F32, tag="aux",
                                      name=f"yb{qc}_{qj}")
                    nc.tensor.matmul(yb[:, 0:256], oT01[:], wp01_r[:, 512:768],
                                     start=True, stop=False)
                    nc.tensor.matmul(yb[:, 0:256], oT2[:], wp2_r[:, 512:768],
                                     start=False, stop=True)
                    y_sb = ypool.tile([128, C], F32, tag="y", name="ysb")
                    nc.vector.tensor_copy(y_sb[:, 0:512], ya[:])
                    nc.vector.tensor_copy(y_sb[:, 512:768], yb[:, 0:256])
                    nc.sync.dma_start(out=y_d[qrows, :], in_=y_sb[:])

            # ---------------- schedule ----------------
            chunk(0)
            chunk(1)
            q_block(0)
            for qc in range(NQC):
                ptsA, ptsB = [], []
                for j in range(8):
                    if qc == 0:
                        if j < 6:
                            chunk(j + 2)
                        v_block(2 * j)
                    a, b = scores_pair(qc, j)
                    ptsA.append(a)
                    ptsB.append(b)
                    if qc > 0 and j == 0:
                        norm_qc(qc - 1)
                    if qc > 0 and j == 2:
                        proj_qc(qc - 1)
                    if qc < NQC - 1 and j == 5:
                        q_block(qc + 1)
                pv_pair_burst(ptsA, ptsB)
                hA, hB = [], []
                for i in range(4):
                    a, b = scores_h2(qc, i)
                    hA.append(a)
                    hB.append(b)
                pv_h2_burst(hA, hB)
            norm_qc(NQC - 1)
            proj_qc(NQC - 1)

    nc.compile()
    return nc


def make_in_maps(x, w_qkv, b_qkv, w_proj):
    """Per-core input dicts. Core c: batch c//4, heads 3*(c%4)+[0..2]."""
    x = np.asarray(x, np.float32)
    w_qkv = np.asarray(w_qkv, np.float32)
    b_qkv = np.asarray(b_qkv, np.float32)
    w_proj = np.asarray(w_proj, np.float32)
    bf = ml_dtypes.bfloat16
    q = lambda h: w_qkv[:, h * 64:(h + 1) * 64]
    k = lambda h: w_qkv[:, C + h * 64:C + (h + 1) * 64]
    v = lambda h: w_qkv[:, 2 * C + h * 64:2 * C + (h + 1) * 64]
    qb = lambda h: b_qkv[h * 64:(h + 1) * 64]
    kb = lambda h: b_qkv[C + h * 64:C + (h + 1) * 64]
    vb = lambda h: b_qkv[2 * C + h * 64:2 * C + (h + 1) * 64]
    in_maps = []
    for c in range(NCORES):
        b = c // 4
        h0 = 3 * (c % 4)
        h1, h2 = h0 + 1, h0 + 2
        w_pack = np.concatenate(
            [q(h0), q(h1), k(h0), k(h1), q(h2), q(h2), k(h2), k(h2)], axis=1)
        bias = np.concatenate(
            [qb(h0), qb(h1), kb(h0), kb(h1), qb(h2), qb(h2), kb(h2), kb(h2)])
        bq_pack = bias.reshape(4, 128).T.copy()
        wv_pack = np.concatenate([v(h0), v(h1), v(h2)], axis=1)
        vb_pack = np.broadcast_to(
            np.concatenate([vb(h0), vb(h1), vb(h2)]), (128, 192))
        wp01 = np.concatenate(
            [w_proj[h0 * 64:(h0 + 1) * 64], w_proj[h1 * 64:(h1 + 1) * 64]],
            axis=0)
        wp2 = w_proj[h2 * 64:(h2 + 1) * 64]
        in_maps.append({
            "x": np.ascontiguousarray(x[b]),
            "w": np.ascontiguousarray(w_pack).astype(bf),
            "wv": np.ascontiguousarray(wv_pack).astype(bf),
            "bq": np.ascontiguousarray(bq_pack, np.float32),
            "vb": np.ascontiguousarray(vb_pack, np.float32),
            "wp01": np.ascontiguousarray(wp01, np.float32),
            "wp2": np.ascontiguousarray(wp2, np.float32),
        })
    return in_maps


_NC_CACHE = []


def _get_program():
    if not _NC_CACHE:
        _NC_CACHE.append(build_program())
    return _NC_CACHE[0]


def run(inputs, trace=False, **kw):
    nc = _get_program()
    in_maps = make_in_maps(inputs["x"], inputs["w_qkv"], inputs["b_qkv"],
                           inputs["w_proj"])
    res = run_bass_kernel_spmd(nc, in_maps, list(range(NCORES)), trace=trace, **kw)
    b_proj = np.asarray(inputs["b_proj"], np.float32)
    out = np.zeros((B, N, C), np.float32)
    for c in range(NCORES):
        out[c // 4] += res.results[c]["y"]
    out += b_proj[None, None, :]
    return out.astype(np.float32), res


def kernel(**inputs):
    out, _ = run(inputs)
    return out
